# revision 37
# baseline (speedup 1.0000x reference)
"""Trainium2 Bass kernel for BoundaryPredictor2 (B=4, L=1024, D=512, H=8).

Sharding: 8 cores = 4 batch rows x 2 token-halves (512 tokens each).

Phase C-first (feature-major activations throughout, zero PE transposes in
the main path):
  l2norm -> layernorm -> V matmul + fused scores (host-precomputed
  Wpk^T@q) -> z = V*e; pairwise AllGather of z fires early and hides under
  the boundary chain.
Phase A (boundary chain, fp32r matmuls; cos reduction in true fp32;
  empirically zero boundary flips vs the fp64 chain):
  MLP(gelu) -> residual l2norm -> qh/kh -> adjacent cos -> hard.
  The cross-half "straddle" pair is handled by shipping each core's first
  kh / last qh column through the hard-AllGather and patching locally.
Phase B: masks + cumsum (tensor_tensor_scan) -> segment ids.
  (The reference's emergency boundary provably never changes seg ids.)
Pooling: one-hot A blocks x z -> own 512 segment slots in PSUM, fused with
  guarded 1/denom, normalize, PE-transpose and @ Wpo^T -> out [512, 512].

The program is uniform SPMD; all per-core differences (token slice, masks,
half-shift) are host-fed data.
"""

from contextlib import ExitStack

import numpy as np

import concourse.bass as bass
import concourse.tile as tile
from concourse import bacc, mybir
from concourse.bass_utils import run_bass_kernel_spmd

FP = mybir.dt.float32
FR = mybir.dt.float32r
F16 = mybir.dt.float16
AF = mybir.ActivationFunctionType
OP = mybir.AluOpType
AX = mybir.AxisListType

N_CORES = 8
B, L, D = 4, 1024, 512
H, HD = 8, 64
TPC = 512           # own tokens per core
NCH = D // 128      # feature chunks (4)
SCALE = HD ** -0.5
ZF = D + H          # z feature width (512 V cols + 8 e cols)
CCW = 3 * TPC       # exchange-1 payload: hard row + kh col0 + qh col511
TG = ((0, 256), (256, 256))   # phase-A token pipeline groups

_CACHE = {}


def _emit(nc, tc, prm, out):
    ctx = ExitStack()
    cpool = ctx.enter_context(tc.tile_pool(name="consts", bufs=1))
    wpool = ctx.enter_context(tc.tile_pool(name="weights", bufs=1))
    apool = ctx.enter_context(tc.tile_pool(name="acts", bufs=1))
    spool = ctx.enter_context(tc.tile_pool(name="scratch", bufs=2))
    rpool = ctx.enter_context(tc.tile_pool(name="rows", bufs=1))
    psm = ctx.enter_context(tc.tile_pool(name="psm", bufs=3, space="PSUM"))
    pbig = ctx.enter_context(tc.tile_pool(name="pbig", bufs=4, space="PSUM"))
    dpool = ctx.enter_context(tc.tile_pool(name="dram", bufs=1, space="DRAM"))

    def ps_small():
        return psm.tile([128, 512], FP, tag="ps1", name="ps1")

    def ps_big():
        return pbig.tile([128, 512], FP, tag="pb", name="pb")

    def dma(dst, src):
        nc.sync.dma_start(out=dst, in_=src)

    def load(pool, name, shape, dt=FP, tag=None):
        t = pool.tile(list(shape), dt, tag=tag or name, name=name)
        dma(t[:], prm[name])
        return t

    # ---- input DMAs in consumption order ----
    xT = [apool.tile([128, TPC], FP, tag=f"xT{c}", name=f"xT{c}")
          for c in range(NCH)]
    for c in range(NCH):
        dma(xT[c][:], prm["hT"][c * 128:(c + 1) * 128, :])
    ones128 = load(cpool, "ones128", (128, 1))
    ones_r = load(cpool, "ones_r", (1, 128))
    ident = load(cpool, "ident", (128, 128))
    simb = load(cpool, "simb", (1, 1))
    wkeff = cpool.tile([128, NCH * H], FR, tag="wkeff", name="wkeff")
    for c in range(NCH):
        dma(wkeff[:, c * H:(c + 1) * H],
            prm["wkeff"][c * 128:(c + 1) * 128, :])
    lenmask = load(cpool, "lenmask", (128, 4))
    lng = load(cpool, "lng", (128, NCH))
    lnb = load(cpool, "lnb", (128, NCH))
    bias1 = load(cpool, "bias1", (128, NCH))
    bias2 = load(cpool, "bias2", (128, NCH))
    hshift = load(cpool, "hshift", (128, 1))

    wt = {}
    for w in ("wpvt", "w1t", "w2t", "wqt", "wkt", "wpot"):
        wdt = F16 if w == "wpot" else FR
        wt[w] = [wpool.tile([128, D], wdt, tag=f"{w}{c}", name=f"{w}{c}")
                 for c in range(NCH)]
        for c in range(NCH):
            nc.gpsimd.dma_start(out=wt[w][c][:],
                                in_=prm[w][c * 128:(c + 1) * 128, :])

    hbmask = load(cpool, "hbmask", (1, L))
    setm = load(cpool, "setm", (1, L))
    iotaL = cpool.tile([128, L], FP, tag="iotaL", name="iotaL")
    nc.gpsimd.dma_start(out=iotaL[:], in_=prm["iotaL"])

    # FR twins (fp32r matmul operands must be produced as fp32r)
    ones128r = cpool.tile([128, 1], FR, tag="ones128r", name="ones128r")
    nc.vector.tensor_copy(ones128r[:], ones128[:])
    ones_rr = cpool.tile([1, 128], FR, tag="ones_rr", name="ones_rr")
    nc.vector.tensor_copy(ones_rr[:], ones_r[:])
    identr = cpool.tile([128, 128], FR, tag="identr", name="identr")
    nc.vector.tensor_copy(identr[:], ident[:])
    identh = cpool.tile([128, 128], F16, tag="identh", name="identh")
    nc.vector.tensor_copy(identh[:], ident[:])

    # ---- warmup collective: absorbs cross-core rendezvous skew ----
    pairs = [[2 * i, 2 * i + 1] for i in range(N_CORES // 2)]
    wui = dpool.tile([1, 1], FP, tag="wui", name="wui")
    wuo = dpool.tile([2, 1], FP, tag="wuo", name="wuo")
    nc.sync.dma_start(out=wui[:], in_=simb[:])
    nc.gpsimd.collective_compute(
        "AllGather", OP.bypass, replica_groups=pairs,
        ins=[wui.opt()], outs=[wuo.opt()])

    def col(t, c):
        return t[:, c:c + 1]

    # 1/sqrt via ACT Sqrt (<=2ulp) + fast DVE reciprocal (~51ulp):
    # norm-scale errors are multiplicative on cos, so sign-safe.
    def rsqrt_row(dst, src, eps, mode, fr=False, w=TPC):
        sm = rpool.tile([1, TPC], FP, tag="rs_sm", name="rs_sm",
                        bufs=3)[0:1, 0:w]
        nc.vector.tensor_scalar(out=sm, in0=src, scalar1=eps,
                                scalar2=None,
                                op0=(OP.max if mode == "clip" else OP.add))
        sqv = rpool.tile([1, TPC], FP, tag="rs_sq", name="rs_sq",
                         bufs=3)[0:1, 0:w]
        nc.scalar.activation(sqv, sm, AF.Sqrt)
        if fr:
            r0 = rpool.tile([1, TPC], FP, tag="rs_r0", name="rs_r0",
                            bufs=3)[0:1, 0:w]
            nc.vector.reciprocal_approx_fast(r0, sqv)
            nc.vector.tensor_copy(dst, r0)
        else:
            nc.vector.reciprocal_approx_fast(dst, sqv)

    # dst[c] = src[c] * rsqrt(sum_d src^2); token-group pipelined
    def l2norm_fm(src_tiles, dst_tiles, msq_keep=None):
        pss = {}
        for g, (g0, gn) in enumerate(TG):
            ps = ps_small()
            for c in range(NCH):
                sq = spool.tile([128, 256], FR, tag="sq", name="sq", bufs=4)
                nc.vector.tensor_mul(sq[:], src_tiles[c][:, g0:g0 + gn],
                                     src_tiles[c][:, g0:g0 + gn])
                nc.tensor.matmul(ps[0:1, 0:gn], ones128r[:], sq[:],
                                 start=(c == 0), stop=(c == NCH - 1))
            pss[g] = ps
            r = rpool.tile([1, 256], FR, tag="nrm_r", name="nrm_r", bufs=2)
            rsqrt_row(r[0:1, :], ps[0:1, 0:gn], 1e-16, "clip", fr=True,
                      w=gn)
            rb = ps_big()
            nc.tensor.matmul(rb[:, 0:gn], ones_rr[:], r[0:1, :],
                             start=True, stop=True)
            for c in range(NCH):
                nc.vector.tensor_mul(dst_tiles[c][:, g0:g0 + gn],
                                     src_tiles[c][:, g0:g0 + gn],
                                     rb[:, 0:gn])
        if msq_keep is not None:
            nc.vector.tensor_copy(msq_keep[0:1, 0:256], pss[0][0:1, 0:256])
            nc.vector.tensor_copy(msq_keep[0:1, 256:512], pss[1][0:1, 0:256])

    # ---- l2norm of hidden (shared by boundary chain and layernorm) ----
    hn2r = [apool.tile([128, TPC], FR, tag=f"hn2r_{c}", name=f"hn2r_{c}")
            for c in range(NCH)]
    msq = rpool.tile([1, TPC], FP, tag="msq", name="msq")[0:1, :]
    l2norm_fm(xT, hn2r, msq_keep=msq)

    # ---- layernorm -> V + scores -> z; early z AllGather ----
    mups = ps_small()
    for c in range(NCH):
        nc.tensor.matmul(mups[0:1, :], ones128[:], xT[c][:],
                         start=(c == 0), stop=(c == NCH - 1))
    mu = rpool.tile([1, TPC], FP, tag="mu", name="mu")[0:1, :]
    nc.vector.tensor_scalar(out=mu, in0=mups[0:1, :], scalar1=1.0 / D,
                            scalar2=None, op0=OP.mult)
    var = rpool.tile([1, TPC], FP, tag="var", name="var")[0:1, :]
    nc.vector.tensor_scalar(out=var, in0=msq, scalar1=1.0 / D,
                            scalar2=None, op0=OP.mult)
    mu2 = rpool.tile([1, TPC], FP, tag="mu2", name="mu2")[0:1, :]
    nc.vector.tensor_mul(mu2, mu, mu)
    nc.vector.tensor_sub(var, var, mu2)
    rstd = rpool.tile([1, TPC], FP, tag="rstd", name="rstd")[0:1, :]
    rsqrt_row(rstd, var, 1e-5, "add")
    mub = ps_big()
    nc.tensor.matmul(mub[:], ones_r[:], mu, start=True, stop=True)
    rstdb = ps_big()
    nc.tensor.matmul(rstdb[:], ones_r[:], rstd, start=True, stop=True)
    hn = [apool.tile([128, TPC], FR, tag=f"hn_{c}", name=f"hn_{c}")
          for c in range(NCH)]
    for c in range(NCH):
        ht = spool.tile([128, TPC], FP, tag="htmp", name="htmp")
        nc.vector.tensor_sub(ht[:], xT[c][:], mub[:])
        nc.vector.tensor_mul(ht[:], ht[:], rstdb[:])
        nc.vector.tensor_scalar(out=hn[c][:], in0=ht[:],
                                scalar1=col(lng, c), scalar2=col(lnb, c),
                                op0=OP.mult, op1=OP.add)

    z = [apool.tile([128, ZF], F16, tag=f"z_{t}", name=f"z_{t}")
         for t in range(4)]
    for tch in range(4):
        tsl = slice(tch * 128, (tch + 1) * 128)
        scps = ps_small()
        for c in range(NCH):
            nc.tensor.matmul(scps[:, 0:H], hn[c][:, tsl],
                             wkeff[:, c * H:(c + 1) * H],
                             start=(c == 0), stop=(c == NCH - 1))
        e = spool.tile([128, H], FP, tag="e", name="e")
        nc.scalar.activation(e[:], scps[:, 0:H], AF.Exp, scale=SCALE)
        nc.vector.tensor_scalar(out=e[:], in0=e[:],
                                scalar1=lenmask[:, tch:tch + 1], scalar2=None,
                                op0=OP.mult)
        vp = ps_big()
        for c in range(NCH):
            nc.tensor.matmul(vp[:], hn[c][:, tsl], wt["wpvt"][c][:],
                             start=(c == 0), stop=(c == NCH - 1))
        nc.vector.tensor_tensor(
            out=z[tch][:, 0:D].rearrange("p (h d) -> p h d", h=H),
            in0=vp[:].rearrange("p (h d) -> p h d", h=H),
            in1=e[:].broadcast_to([128, H, HD]),
            op=OP.mult)
        nc.vector.tensor_copy(z[tch][:, D:ZF], e[:])

    cczi = dpool.tile([TPC, ZF], F16, tag="cczi", name="cczi")
    cczo = dpool.tile([L, ZF], F16, tag="cczo", name="cczo")
    for t in range(4):
        dma(cczi[t * 128:(t + 1) * 128, :], z[t][:])
    nc.gpsimd.collective_compute(
        "AllGather", OP.bypass, replica_groups=pairs,
        ins=[cczi.opt()], outs=[cczo.opt()])
    zf = []
    for k in range(8):
        tag = f"z_{k}" if k < 4 else f"hn_{k - 4}"
        zk = apool.tile([128, ZF], F16, tag=tag, name=f"zf_{k}")
        dma(zk[:], cczo[k * 128:(k + 1) * 128, :])
        zf.append(zk)

    # ---- boundary chain (fp32r MLP; cos reduction in true fp32) ----
    def linear_fm(w, src_tiles, ech, g, resid=False):
        g0, gn = TG[g]
        ps = ps_big()
        for c in range(NCH):
            nc.tensor.matmul(ps[:, 0:gn],
                             wt[w][c][:, ech * 128:(ech + 1) * 128],
                             src_tiles[c][:, g0:g0 + gn],
                             start=(c == 0), stop=False if resid else
                             (c == NCH - 1))
        if resid:
            nc.tensor.matmul(ps[:, 0:gn], identr[:],
                             hn2r[ech][:, g0:g0 + gn],
                             start=False, stop=True)
        return ps[:, 0:gn]

    t1 = [apool.tile([128, TPC], FR, tag=f"t1_{c}", name=f"t1_{c}")
          for c in range(NCH)]
    for g, (g0, gn) in enumerate(TG):
        for ech in range(NCH):
            ps = linear_fm("w1t", hn2r, ech, g)
            nc.scalar.activation(t1[ech][:, g0:g0 + gn], ps, AF.Gelu,
                                 bias=col(bias1, ech))

    v = [apool.tile([128, TPC], FP, tag=f"v_{c}", name=f"v_{c}")
         for c in range(NCH)]
    for g, (g0, gn) in enumerate(TG):
        for ech in range(NCH):
            ps = linear_fm("w2t", t1, ech, g, resid=True)
            nc.vector.tensor_scalar(out=v[ech][:, g0:g0 + gn], in0=ps,
                                    scalar1=col(bias2, ech), scalar2=None,
                                    op0=OP.add)

    u = [apool.tile([128, TPC], FR, tag=f"u_{c}", name=f"u_{c}")
         for c in range(NCH)]
    l2norm_fm(v, u)

    qh = [apool.tile([128, TPC], FP, tag=f"qh_{c}", name=f"qh_{c}")
          for c in range(NCH)]
    kh = [apool.tile([128, TPC], FP, tag=f"kh_{c}", name=f"kh_{c}")
          for c in range(NCH)]
    for g, (g0, gn) in enumerate(TG):
        for dst, w in ((qh, "wqt"), (kh, "wkt")):
            for ech in range(NCH):
                ps = linear_fm(w, u, ech, g)
                nc.scalar.copy(dst[ech][:, g0:g0 + gn], ps)

    cosps = ps_small()
    for c in range(NCH):
        pr = spool.tile([128, TPC], FP, tag="prod", name="prod")
        nc.vector.tensor_mul(pr[:, 0:TPC - 1], qh[c][:, 0:TPC - 1],
                             kh[c][:, 1:TPC])
        nc.vector.memset(pr[:, TPC - 1:TPC], 0.0)
        nc.tensor.matmul(cosps[0:1, :], ones128[:], pr[:],
                         start=(c == 0), stop=(c == NCH - 1))
    sgn = rpool.tile([1, TPC], FP, tag="sgn", name="sgn")[0:1, :]
    nc.scalar.activation(sgn, cosps[0:1, :], AF.Sign, bias=simb[0:1, 0:1])
    hard = rpool.tile([1, TPC], FP, tag="hard", name="hard")[0:1, :]
    nc.scalar.activation(hard, sgn, AF.Relu, scale=-1.0)

    # ---- exchange 1: hard row + boundary kh/qh columns, pairwise ----
    cc1i = dpool.tile([1, CCW], FP, tag="cc1i", name="cc1i")
    cc1o = dpool.tile([2, CCW], FP, tag="cc1o", name="cc1o")
    nc.sync.dma_start(out=cc1i[0:1, 0:TPC], in_=hard)
    for c in range(NCH):
        dma(cc1i[0:1, TPC + c * 128:TPC + (c + 1) * 128], kh[c][:, 0:1])
        dma(cc1i[0:1, 2 * TPC + c * 128:2 * TPC + (c + 1) * 128],
            qh[c][:, TPC - 1:TPC])
    nc.gpsimd.collective_compute(
        "AllGather", OP.bypass, replica_groups=pairs,
        ins=[cc1i.opt()], outs=[cc1o.opt()])
    hardf = rpool.tile([1, L], FP, tag="hardf", name="hardf")[0:1, :]
    dma(hardf[:, 0:TPC], cc1o[0:1, 0:TPC])
    dma(hardf[:, TPC:L], cc1o[1:2, 0:TPC])
    # straddle pair (global 511, 512): qh_last of rank0 . kh_first of rank1
    strq = rpool.tile([1, TPC], FP, tag="strq", name="strq")[0:1, :]
    dma(strq, cc1o[0:1, 2 * TPC:3 * TPC])
    strk = rpool.tile([1, TPC], FP, tag="strk", name="strk")[0:1, :]
    dma(strk, cc1o[1:2, TPC:2 * TPC])
    nc.vector.tensor_mul(strq, strq, strk)
    scos = rpool.tile([1, 1], FP, tag="scos", name="scos")
    nc.vector.reduce_sum(scos[:], strq, axis=AX.X)
    nc.scalar.activation(scos[:], scos[:], AF.Sign, bias=simb[0:1, 0:1])
    nc.scalar.activation(hardf[0:1, TPC - 1:TPC], scos[:], AF.Relu,
                         scale=-1.0)

    # ---- seg ids ----
    hb = rpool.tile([1, L], FP, tag="hb", name="hb")[0:1, :]
    nc.vector.tensor_mul(hb, hardf, hbmask[:])
    nc.vector.tensor_max(hb, hb, setm[:])
    csum = rpool.tile([1, L], FP, tag="hardf", name="csum")[0:1, :]
    nc.vector.tensor_tensor_scan(csum, hb, hb, 0.0, OP.add, OP.bypass)
    segr = rpool.tile([1, L], FP, tag="segr", name="segr")[0:1, :]
    nc.vector.tensor_sub(segr, csum, hb)
    segb = dpool.tile([1, L], FP, tag="segb", name="segb")
    dma(segb[:], segr)
    segc = rpool.tile([128, 8], FP, tag="segc", name="segc")
    dma(segc[:], segb[:].rearrange("a (c p) -> (a p) c", p=128))
    sego = rpool.tile([128, 8], FP, tag="sego", name="sego")
    nc.vector.tensor_scalar(out=sego[:], in0=segc[:],
                            scalar1=hshift[:, 0:1], scalar2=None,
                            op0=OP.subtract)

    # ---- pooling into own s-half + normalize + Wpo, fused per s-chunk ----
    Af = [apool.tile([128, TPC], F16,
                     tag=f"xT{k}" if k < 4 else f"t1_{k - 4}",
                     name=f"Af_{k}") for k in range(8)]
    for k in range(8):
        nc.vector.tensor_scalar(out=Af[k][:], in0=iotaL[:, 0:TPC],
                                scalar1=sego[:, k:k + 1], scalar2=None,
                                op0=OP.is_equal)
    pT = [apool.tile([128, TPC], F16, tag=f"kh_{c}", name=f"pT_{c}")
          for c in range(NCH)]
    for j in range(4):
        psA = ps_big()
        psB = ps_small()
        ks = list(range(j, 8))
        for i, k in enumerate(ks):
            jsl = slice(j * 128, (j + 1) * 128)
            nc.tensor.matmul(psA[:], Af[k][:, jsl], zf[k][:, 0:D],
                             start=(i == 0), stop=(i == len(ks) - 1))
            nc.tensor.matmul(psB[:, 0:H], Af[k][:, jsl], zf[k][:, D:ZF],
                             start=(i == 0), stop=(i == len(ks) - 1))
        dd = spool.tile([128, H], FP, tag="dd", name="dd")
        nc.vector.tensor_scalar(out=dd[:], in0=psB[:, 0:H], scalar1=0.0,
                                scalar2=None, op0=OP.is_equal)
        nc.vector.tensor_add(dd[:], dd[:], psB[:, 0:H])
        rec = spool.tile([128, H], FP, tag="rec", name="rec")
        nc.vector.reciprocal_approx_fast(rec[:], dd[:])
        pn = spool.tile([128, D], F16, tag="pn", name="pn")
        nc.vector.tensor_tensor(
            out=pn[:].rearrange("p (h d) -> p h d", h=H),
            in0=psA[:].rearrange("p (h d) -> p h d", h=H),
            in1=rec[:].broadcast_to([128, H, HD]),
            op=OP.mult)
        for c in range(NCH):
            psT = ps_small().bitcast(F16)
            nc.tensor.transpose(psT[:, 0:128], pn[:, c * 128:(c + 1) * 128],
                                identh[:])
            nc.vector.tensor_copy(pT[c][:, j * 128:(j + 1) * 128],
                                  psT[:, 0:128])
        ps = ps_big()
        for c in range(NCH):
            nc.tensor.matmul(ps[:],
                             pT[c][:, j * 128:(j + 1) * 128],
                             wt["wpot"][c][:],
                             start=(c == 0), stop=(c == NCH - 1))
        osb = spool.tile([128, D], FP, tag="osb", name="osb")
        nc.vector.tensor_copy(osb[:], ps[:])
        dma(out[j * 128:(j + 1) * 128, :], osb[:])

    ctx.close()


def _build():
    if "nc" in _CACHE:
        return _CACHE["nc"]
    nc = bacc.Bacc("TRN2", target_bir_lowering=False, debug=False,
                   num_devices=N_CORES)
    names = {
        "hT": (D, TPC), "w1t": (D, D), "w2t": (D, D), "wqt": (D, D),
        "wkt": (D, D), "wpvt": (D, D), "wpot": (D, D),
        "ones128": (128, 1), "ones_r": (1, 128), "ident": (128, 128),
        "iotaL": (128, L), "simb": (1, 1), "hbmask": (1, L),
        "setm": (1, L), "lenmask": (128, 4),
        "wkeff": (D, H), "bias1": (128, NCH), "bias2": (128, NCH),
        "lng": (128, NCH), "lnb": (128, NCH), "hshift": (128, 1),
    }
    _fr = {"w1t", "w2t", "wqt", "wkt", "wpvt", "wkeff"}
    prm = {}
    for k, sh in names.items():
        dt = FR if k in _fr else (F16 if k == "wpot" else FP)
        prm[k] = nc.dram_tensor(k, list(sh), dt, kind="ExternalInput").ap()
    out = nc.dram_tensor("out", [TPC, D], FP, kind="ExternalOutput").ap()
    with tile.TileContext(nc) as tc:
        _emit(nc, tc, prm, out)
    nc.compile()
    _CACHE["nc"] = nc
    return nc


def _host_prep(inputs):
    f32 = np.float32
    hidden = np.asarray(inputs["hidden"], f32)
    lengths = np.asarray(inputs["lengths"], f32)
    consts = {
        "ones128": np.ones((128, 1), f32),
        "ones_r": np.ones((1, 128), f32),
        "ident": np.eye(128, dtype=f32),
        "iotaL": np.tile(np.arange(L, dtype=f32), (128, 1)),
        "simb": np.asarray(inputs["sim_bias"], f32).reshape(1, 1),
        "wkeff": np.ascontiguousarray(
            (np.asarray(inputs["Wpk"], np.float64).T.reshape(D, H, HD)
             * np.asarray(inputs["learned_query"],
                          np.float64).reshape(H, HD)[None]
             ).sum(-1).astype(f32)),
        "bias1": np.ascontiguousarray(
            np.asarray(inputs["b1"], f32).reshape(NCH, 128).T),
        "bias2": np.ascontiguousarray(
            np.asarray(inputs["b2"], f32).reshape(NCH, 128).T),
        "lng": np.ascontiguousarray(
            np.asarray(inputs["ln_g"], f32).reshape(NCH, 128).T),
        "lnb": np.ascontiguousarray(
            np.asarray(inputs["ln_b"], f32).reshape(NCH, 128).T),
    }
    for k, w in (("w1t", "W1"), ("w2t", "W2"), ("wqt", "Wq"), ("wkt", "Wk"),
                 ("wpvt", "Wpv")):
        consts[k] = np.ascontiguousarray(np.asarray(inputs[w], f32).T)
    consts["wpot"] = np.ascontiguousarray(
        np.asarray(inputs["Wpo"], np.float16).T)

    actual = (lengths * f32(L + 1)).astype(np.int32)
    valid = np.clip(actual - 1, 0, L)
    cut = (lengths * f32(L)).astype(np.int32)
    pos = np.arange(L)

    in_maps = []
    for c in range(N_CORES):
        b, h = c // 2, c % 2
        tok0 = h * TPC
        hT = np.ascontiguousarray(hidden[b, tok0:tok0 + TPC, :].T)
        vm = ((pos < valid[b]) & (pos < L - 1)).astype(f32)[None, :]
        st = np.zeros((1, L), f32)
        if valid[b] < L:
            st[0, valid[b]] = 1.0
        lm = np.zeros((128, 4), f32)
        for tch in range(4):
            g = tok0 + tch * 128 + np.arange(128)
            lm[:, tch] = (g < cut[b]).astype(f32)
        m = dict(consts)
        m.update({
            "hT": hT, "lenmask": lm, "hbmask": vm, "setm": st,
            "hshift": np.full((128, 1), 512.0 * h, f32),
        })
        in_maps.append(m)
    return in_maps


def kernel(**inputs):
    nc = _build()
    in_maps = _host_prep(inputs)
    res = run_bass_kernel_spmd(nc, in_maps, list(range(N_CORES)))
    out = np.empty((B, L, D), np.float32)
    for c in range(N_CORES):
        b, h = c // 2, c % 2
        out[b, h * TPC:(h + 1) * TPC, :] = res.results[c]["out"]
    return out


# revision 38
# speedup vs baseline: 1.0710x; 1.0710x over previous
"""Trainium2 Bass kernel for BoundaryPredictor2 (B=4, L=1024, D=512, H=8).

Sharding: 8 cores = 4 batch rows x 2 token-halves (512 tokens each).

Phase C-first (feature-major activations throughout, zero PE transposes in
the main path):
  l2norm -> layernorm -> V matmul + fused scores (host-precomputed
  Wpk^T@q) -> z = V*e; pairwise AllGather of z fires early and hides under
  the boundary chain.
Phase A (boundary chain, fp32r matmuls; cos reduction in true fp32;
  empirically zero boundary flips vs the fp64 chain):
  MLP(gelu) -> residual l2norm -> qh/kh -> adjacent cos -> hard.
  The cross-half "straddle" pair is handled by shipping each core's first
  kh / last qh column through the hard-AllGather and patching locally.
Phase B: masks + cumsum (tensor_tensor_scan) -> segment ids.
  (The reference's emergency boundary provably never changes seg ids.)
Pooling: one-hot A blocks x z -> own 512 segment slots in PSUM, fused with
  guarded 1/denom, normalize, PE-transpose and @ Wpo^T -> out [512, 512].

The program is uniform SPMD; all per-core differences (token slice, masks,
half-shift) are host-fed data.
"""

from contextlib import ExitStack

import numpy as np

import concourse.bass as bass
import concourse.tile as tile
from concourse import bacc, mybir
from concourse.bass_utils import run_bass_kernel_spmd

FP = mybir.dt.float32
FR = mybir.dt.float32r
F16 = mybir.dt.float16
AF = mybir.ActivationFunctionType
OP = mybir.AluOpType
AX = mybir.AxisListType

N_CORES = 8
B, L, D = 4, 1024, 512
H, HD = 8, 64
TPC = 512           # own tokens per core
NCH = D // 128      # feature chunks (4)
SCALE = HD ** -0.5
ZF = D + H          # z feature width (512 V cols + 8 e cols)
CCW = 3 * TPC       # exchange-1 payload: hard row + kh col0 + qh col511
TG = ((0, 256), (256, 256))   # phase-A token pipeline groups

_CACHE = {}


def _emit(nc, tc, prm, out):
    ctx = ExitStack()
    cpool = ctx.enter_context(tc.tile_pool(name="consts", bufs=1))
    wpool = ctx.enter_context(tc.tile_pool(name="weights", bufs=1))
    apool = ctx.enter_context(tc.tile_pool(name="acts", bufs=1))
    spool = ctx.enter_context(tc.tile_pool(name="scratch", bufs=2))
    rpool = ctx.enter_context(tc.tile_pool(name="rows", bufs=1))
    psm = ctx.enter_context(tc.tile_pool(name="psm", bufs=3, space="PSUM"))
    pbig = ctx.enter_context(tc.tile_pool(name="pbig", bufs=4, space="PSUM"))
    dpool = ctx.enter_context(tc.tile_pool(name="dram", bufs=1, space="DRAM"))

    def ps_small():
        return psm.tile([128, 512], FP, tag="ps1", name="ps1")

    def ps_big():
        return pbig.tile([128, 512], FP, tag="pb", name="pb")

    def dma(dst, src):
        nc.sync.dma_start(out=dst, in_=src)

    def load(pool, name, shape, dt=FP, tag=None):
        t = pool.tile(list(shape), dt, tag=tag or name, name=name)
        dma(t[:], prm[name])
        return t

    # ---- input DMAs in consumption order ----
    xT = [apool.tile([128, TPC], FP, tag=f"xT{c}", name=f"xT{c}")
          for c in range(NCH)]
    for c in range(NCH):
        dma(xT[c][:], prm["hT"][c * 128:(c + 1) * 128, :])
    ones128 = load(cpool, "ones128", (128, 1))
    ones_r = load(cpool, "ones_r", (1, 128))
    ident = load(cpool, "ident", (128, 128))
    simb = load(cpool, "simb", (1, 1))
    wkeff = cpool.tile([128, NCH * H], FR, tag="wkeff", name="wkeff")
    for c in range(NCH):
        dma(wkeff[:, c * H:(c + 1) * H],
            prm["wkeff"][c * 128:(c + 1) * 128, :])
    lenmask = load(cpool, "lenmask", (128, 4))
    lng = load(cpool, "lng", (128, NCH))
    lnb = load(cpool, "lnb", (128, NCH))
    bias1 = load(cpool, "bias1", (128, NCH))
    bias2 = load(cpool, "bias2", (128, NCH))
    hshift = load(cpool, "hshift", (128, 1))

    wt = {}
    for w in ("wpvt", "w1t", "w2t", "wqt", "wkt", "wpot"):
        wdt = F16 if w == "wpot" else FR
        wt[w] = [wpool.tile([128, D], wdt, tag=f"{w}{c}", name=f"{w}{c}")
                 for c in range(NCH)]
        for c in range(NCH):
            dma(wt[w][c][:], prm[w][c * 128:(c + 1) * 128, :])

    hbmask = load(cpool, "hbmask", (1, L))
    setm = load(cpool, "setm", (1, L))
    iotaL = load(cpool, "iotaL", (128, L))

    # FR twins (fp32r matmul operands must be produced as fp32r)
    ones128r = cpool.tile([128, 1], FR, tag="ones128r", name="ones128r")
    nc.vector.tensor_copy(ones128r[:], ones128[:])
    ones_rr = cpool.tile([1, 128], FR, tag="ones_rr", name="ones_rr")
    nc.vector.tensor_copy(ones_rr[:], ones_r[:])
    identr = cpool.tile([128, 128], FR, tag="identr", name="identr")
    nc.vector.tensor_copy(identr[:], ident[:])
    identh = cpool.tile([128, 128], F16, tag="identh", name="identh")
    nc.vector.tensor_copy(identh[:], ident[:])

    # ---- warmup collective: absorbs cross-core rendezvous skew ----
    pairs = [[2 * i, 2 * i + 1] for i in range(N_CORES // 2)]
    wui = dpool.tile([1, 1], FP, tag="wui", name="wui")
    wuo = dpool.tile([2, 1], FP, tag="wuo", name="wuo")
    nc.sync.dma_start(out=wui[:], in_=simb[:])
    nc.gpsimd.collective_compute(
        "AllGather", OP.bypass, replica_groups=pairs,
        ins=[wui.opt()], outs=[wuo.opt()])

    def col(t, c):
        return t[:, c:c + 1]

    # 1/sqrt via ACT Sqrt (<=2ulp) + fast DVE reciprocal (~51ulp):
    # norm-scale errors are multiplicative on cos, so sign-safe.
    def rsqrt_row(dst, src, eps, mode, fr=False, w=TPC):
        sm = rpool.tile([1, TPC], FP, tag="rs_sm", name="rs_sm",
                        bufs=3)[0:1, 0:w]
        nc.vector.tensor_scalar(out=sm, in0=src, scalar1=eps,
                                scalar2=None,
                                op0=(OP.max if mode == "clip" else OP.add))
        sqv = rpool.tile([1, TPC], FP, tag="rs_sq", name="rs_sq",
                         bufs=3)[0:1, 0:w]
        nc.scalar.activation(sqv, sm, AF.Sqrt)
        if fr:
            r0 = rpool.tile([1, TPC], FP, tag="rs_r0", name="rs_r0",
                            bufs=3)[0:1, 0:w]
            nc.vector.reciprocal_approx_fast(r0, sqv)
            nc.vector.tensor_copy(dst, r0)
        else:
            nc.vector.reciprocal_approx_fast(dst, sqv)

    # dst[c] = src[c] * rsqrt(sum_d src^2); token-group pipelined
    def l2norm_fm(src_tiles, dst_tiles, msq_keep=None):
        pss = {}
        for g, (g0, gn) in enumerate(TG):
            ps = ps_small()
            for c in range(NCH):
                sq = spool.tile([128, 256], FR, tag="sq", name="sq", bufs=4)
                nc.vector.tensor_mul(sq[:], src_tiles[c][:, g0:g0 + gn],
                                     src_tiles[c][:, g0:g0 + gn])
                nc.tensor.matmul(ps[0:1, 0:gn], ones128r[:], sq[:],
                                 start=(c == 0), stop=(c == NCH - 1))
            pss[g] = ps
            r = rpool.tile([1, 256], FR, tag="nrm_r", name="nrm_r", bufs=2)
            rsqrt_row(r[0:1, :], ps[0:1, 0:gn], 1e-16, "clip", fr=True,
                      w=gn)
            rb = ps_big()
            nc.tensor.matmul(rb[:, 0:gn], ones_rr[:], r[0:1, :],
                             start=True, stop=True)
            for c in range(NCH):
                nc.vector.tensor_mul(dst_tiles[c][:, g0:g0 + gn],
                                     src_tiles[c][:, g0:g0 + gn],
                                     rb[:, 0:gn])
        if msq_keep is not None:
            nc.vector.tensor_copy(msq_keep[0:1, 0:256], pss[0][0:1, 0:256])
            nc.vector.tensor_copy(msq_keep[0:1, 256:512], pss[1][0:1, 0:256])

    # ---- l2norm of hidden (shared by boundary chain and layernorm) ----
    hn2r = [apool.tile([128, TPC], FR, tag=f"hn2r_{c}", name=f"hn2r_{c}")
            for c in range(NCH)]
    msq = rpool.tile([1, TPC], FP, tag="msq", name="msq")[0:1, :]
    l2norm_fm(xT, hn2r, msq_keep=msq)

    # ---- layernorm -> V + scores -> z; early z AllGather ----
    mups = ps_small()
    for c in range(NCH):
        nc.tensor.matmul(mups[0:1, :], ones128[:], xT[c][:],
                         start=(c == 0), stop=(c == NCH - 1))
    mu = rpool.tile([1, TPC], FP, tag="mu", name="mu")[0:1, :]
    nc.vector.tensor_scalar(out=mu, in0=mups[0:1, :], scalar1=1.0 / D,
                            scalar2=None, op0=OP.mult)
    var = rpool.tile([1, TPC], FP, tag="var", name="var")[0:1, :]
    nc.vector.tensor_scalar(out=var, in0=msq, scalar1=1.0 / D,
                            scalar2=None, op0=OP.mult)
    mu2 = rpool.tile([1, TPC], FP, tag="mu2", name="mu2")[0:1, :]
    nc.vector.tensor_mul(mu2, mu, mu)
    nc.vector.tensor_sub(var, var, mu2)
    rstd = rpool.tile([1, TPC], FP, tag="rstd", name="rstd")[0:1, :]
    rsqrt_row(rstd, var, 1e-5, "add")
    mub = ps_big()
    nc.tensor.matmul(mub[:], ones_r[:], mu, start=True, stop=True)
    rstdb = ps_big()
    nc.tensor.matmul(rstdb[:], ones_r[:], rstd, start=True, stop=True)
    hn = [apool.tile([128, TPC], FR, tag=f"hn_{c}", name=f"hn_{c}")
          for c in range(NCH)]
    for c in range(NCH):
        ht = spool.tile([128, TPC], FP, tag="htmp", name="htmp")
        nc.vector.tensor_sub(ht[:], xT[c][:], mub[:])
        nc.vector.tensor_mul(ht[:], ht[:], rstdb[:])
        nc.vector.tensor_scalar(out=hn[c][:], in0=ht[:],
                                scalar1=col(lng, c), scalar2=col(lnb, c),
                                op0=OP.mult, op1=OP.add)

    z = [apool.tile([128, ZF], F16, tag=f"z_{t}", name=f"z_{t}")
         for t in range(4)]
    for tch in range(4):
        tsl = slice(tch * 128, (tch + 1) * 128)
        scps = ps_small()
        for c in range(NCH):
            nc.tensor.matmul(scps[:, 0:H], hn[c][:, tsl],
                             wkeff[:, c * H:(c + 1) * H],
                             start=(c == 0), stop=(c == NCH - 1))
        e = spool.tile([128, H], FP, tag="e", name="e")
        nc.scalar.activation(e[:], scps[:, 0:H], AF.Exp, scale=SCALE)
        nc.vector.tensor_scalar(out=e[:], in0=e[:],
                                scalar1=lenmask[:, tch:tch + 1], scalar2=None,
                                op0=OP.mult)
        vp = ps_big()
        for c in range(NCH):
            nc.tensor.matmul(vp[:], hn[c][:, tsl], wt["wpvt"][c][:],
                             start=(c == 0), stop=(c == NCH - 1))
        nc.vector.tensor_tensor(
            out=z[tch][:, 0:D].rearrange("p (h d) -> p h d", h=H),
            in0=vp[:].rearrange("p (h d) -> p h d", h=H),
            in1=e[:].broadcast_to([128, H, HD]),
            op=OP.mult)
        nc.vector.tensor_copy(z[tch][:, D:ZF], e[:])

    czi = [dpool.tile([256, ZF], F16, tag=f"czi{i}", name=f"czi{i}")
           for i in range(2)]
    czo = [dpool.tile([TPC, ZF], F16, tag=f"czo{i}", name=f"czo{i}")
           for i in range(2)]
    for i in range(2):
        for t in range(2):
            dma(czi[i][t * 128:(t + 1) * 128, :], z[2 * i + t][:])
        nc.gpsimd.collective_compute(
            "AllGather", OP.bypass, replica_groups=pairs,
            ins=[czi[i].opt()], outs=[czo[i].opt()])
    zf = [None] * 8
    for k, (i, row) in enumerate(((0, 0), (0, 1), (1, 0), (1, 1),
                                  (0, 2), (0, 3), (1, 2), (1, 3))):
        tag = f"z_{k}" if k < 4 else f"hn_{k - 4}"
        zk = apool.tile([128, ZF], F16, tag=tag, name=f"zf_{k}")
        dma(zk[:], czo[i][row * 128:(row + 1) * 128, :])
        zf[k] = zk

    # ---- boundary chain (fp32r MLP; cos reduction in true fp32) ----
    def linear_fm(w, src_tiles, ech, g, resid=False):
        g0, gn = TG[g]
        ps = ps_big()
        for c in range(NCH):
            nc.tensor.matmul(ps[:, 0:gn],
                             wt[w][c][:, ech * 128:(ech + 1) * 128],
                             src_tiles[c][:, g0:g0 + gn],
                             start=(c == 0), stop=False if resid else
                             (c == NCH - 1))
        if resid:
            nc.tensor.matmul(ps[:, 0:gn], identr[:],
                             hn2r[ech][:, g0:g0 + gn],
                             start=False, stop=True)
        return ps[:, 0:gn]

    t1 = [apool.tile([128, TPC], FR, tag=f"t1_{c}", name=f"t1_{c}")
          for c in range(NCH)]
    for g, (g0, gn) in enumerate(TG):
        for ech in range(NCH):
            ps = linear_fm("w1t", hn2r, ech, g)
            nc.scalar.activation(t1[ech][:, g0:g0 + gn], ps, AF.Gelu,
                                 bias=col(bias1, ech))

    v = [apool.tile([128, TPC], FP, tag=f"v_{c}", name=f"v_{c}")
         for c in range(NCH)]
    for g, (g0, gn) in enumerate(TG):
        for ech in range(NCH):
            ps = linear_fm("w2t", t1, ech, g, resid=True)
            nc.vector.tensor_scalar(out=v[ech][:, g0:g0 + gn], in0=ps,
                                    scalar1=col(bias2, ech), scalar2=None,
                                    op0=OP.add)

    u = [apool.tile([128, TPC], FR, tag=f"u_{c}", name=f"u_{c}")
         for c in range(NCH)]
    l2norm_fm(v, u)

    qh = [apool.tile([128, TPC], FP, tag=f"qh_{c}", name=f"qh_{c}")
          for c in range(NCH)]
    kh = [apool.tile([128, TPC], FP, tag=f"kh_{c}", name=f"kh_{c}")
          for c in range(NCH)]
    for g, (g0, gn) in enumerate(TG):
        for dst, w in ((qh, "wqt"), (kh, "wkt")):
            for ech in range(NCH):
                ps = linear_fm(w, u, ech, g)
                nc.scalar.copy(dst[ech][:, g0:g0 + gn], ps)

    cosps = ps_small()
    for c in range(NCH):
        pr = spool.tile([128, TPC], FP, tag="prod", name="prod")
        nc.vector.tensor_mul(pr[:, 0:TPC - 1], qh[c][:, 0:TPC - 1],
                             kh[c][:, 1:TPC])
        nc.vector.memset(pr[:, TPC - 1:TPC], 0.0)
        nc.tensor.matmul(cosps[0:1, :], ones128[:], pr[:],
                         start=(c == 0), stop=(c == NCH - 1))
    sgn = rpool.tile([1, TPC], FP, tag="sgn", name="sgn")[0:1, :]
    nc.scalar.activation(sgn, cosps[0:1, :], AF.Sign, bias=simb[0:1, 0:1])
    hard = rpool.tile([1, TPC], FP, tag="hard", name="hard")[0:1, :]
    nc.scalar.activation(hard, sgn, AF.Relu, scale=-1.0)

    # ---- exchange 1: hard row + boundary kh/qh columns, pairwise ----
    cc1i = dpool.tile([1, CCW], FP, tag="cc1i", name="cc1i")
    cc1o = dpool.tile([2, CCW], FP, tag="cc1o", name="cc1o")
    nc.sync.dma_start(out=cc1i[0:1, 0:TPC], in_=hard)
    for c in range(NCH):
        dma(cc1i[0:1, TPC + c * 128:TPC + (c + 1) * 128], kh[c][:, 0:1])
        dma(cc1i[0:1, 2 * TPC + c * 128:2 * TPC + (c + 1) * 128],
            qh[c][:, TPC - 1:TPC])
    nc.gpsimd.collective_compute(
        "AllGather", OP.bypass, replica_groups=pairs,
        ins=[cc1i.opt()], outs=[cc1o.opt()])
    hardf = rpool.tile([1, L], FP, tag="hardf", name="hardf")[0:1, :]
    dma(hardf[:, 0:TPC], cc1o[0:1, 0:TPC])
    dma(hardf[:, TPC:L], cc1o[1:2, 0:TPC])
    # straddle pair (global 511, 512): qh_last of rank0 . kh_first of rank1
    strq = rpool.tile([1, TPC], FP, tag="strq", name="strq")[0:1, :]
    dma(strq, cc1o[0:1, 2 * TPC:3 * TPC])
    strk = rpool.tile([1, TPC], FP, tag="strk", name="strk")[0:1, :]
    dma(strk, cc1o[1:2, TPC:2 * TPC])
    nc.vector.tensor_mul(strq, strq, strk)
    scos = rpool.tile([1, 1], FP, tag="scos", name="scos")
    nc.vector.reduce_sum(scos[:], strq, axis=AX.X)
    nc.scalar.activation(scos[:], scos[:], AF.Sign, bias=simb[0:1, 0:1])
    nc.scalar.activation(hardf[0:1, TPC - 1:TPC], scos[:], AF.Relu,
                         scale=-1.0)

    # ---- seg ids ----
    hb = rpool.tile([1, L], FP, tag="hb", name="hb")[0:1, :]
    nc.vector.tensor_mul(hb, hardf, hbmask[:])
    nc.vector.tensor_max(hb, hb, setm[:])
    csum = rpool.tile([1, L], FP, tag="hardf", name="csum")[0:1, :]
    nc.vector.tensor_tensor_scan(csum, hb, hb, 0.0, OP.add, OP.bypass)
    segr = rpool.tile([1, L], FP, tag="segr", name="segr")[0:1, :]
    nc.vector.tensor_sub(segr, csum, hb)
    segb = dpool.tile([1, L], FP, tag="segb", name="segb")
    dma(segb[:], segr)
    segc = rpool.tile([128, 8], FP, tag="segc", name="segc")
    dma(segc[:], segb[:].rearrange("a (c p) -> (a p) c", p=128))
    sego = rpool.tile([128, 8], FP, tag="sego", name="sego")
    nc.vector.tensor_scalar(out=sego[:], in0=segc[:],
                            scalar1=hshift[:, 0:1], scalar2=None,
                            op0=OP.subtract)

    # ---- pooling into own s-half + normalize + Wpo, fused per s-chunk ----
    Af = [apool.tile([128, TPC], F16,
                     tag=f"xT{k}" if k < 4 else f"t1_{k - 4}",
                     name=f"Af_{k}") for k in range(8)]
    for k in range(8):
        nc.vector.tensor_scalar(out=Af[k][:], in0=iotaL[:, 0:TPC],
                                scalar1=sego[:, k:k + 1], scalar2=None,
                                op0=OP.is_equal)
    pT = [apool.tile([128, TPC], F16, tag=f"kh_{c}", name=f"pT_{c}")
          for c in range(NCH)]
    for j in range(4):
        psA = ps_big()
        psB = ps_small()
        ks = list(range(j, 8))
        for i, k in enumerate(ks):
            jsl = slice(j * 128, (j + 1) * 128)
            nc.tensor.matmul(psA[:], Af[k][:, jsl], zf[k][:, 0:D],
                             start=(i == 0), stop=(i == len(ks) - 1))
            nc.tensor.matmul(psB[:, 0:H], Af[k][:, jsl], zf[k][:, D:ZF],
                             start=(i == 0), stop=(i == len(ks) - 1))
        dd = spool.tile([128, H], FP, tag="dd", name="dd")
        nc.vector.tensor_scalar(out=dd[:], in0=psB[:, 0:H], scalar1=0.0,
                                scalar2=None, op0=OP.is_equal)
        nc.vector.tensor_add(dd[:], dd[:], psB[:, 0:H])
        rec = spool.tile([128, H], FP, tag="rec", name="rec")
        nc.vector.reciprocal_approx_fast(rec[:], dd[:])
        pn = spool.tile([128, D], F16, tag="pn", name="pn")
        nc.vector.tensor_tensor(
            out=pn[:].rearrange("p (h d) -> p h d", h=H),
            in0=psA[:].rearrange("p (h d) -> p h d", h=H),
            in1=rec[:].broadcast_to([128, H, HD]),
            op=OP.mult)
        for c in range(NCH):
            psT = ps_small().bitcast(F16)
            nc.tensor.transpose(psT[:, 0:128], pn[:, c * 128:(c + 1) * 128],
                                identh[:])
            nc.vector.tensor_copy(pT[c][:, j * 128:(j + 1) * 128],
                                  psT[:, 0:128])
        ps = ps_big()
        for c in range(NCH):
            nc.tensor.matmul(ps[:],
                             pT[c][:, j * 128:(j + 1) * 128],
                             wt["wpot"][c][:],
                             start=(c == 0), stop=(c == NCH - 1))
        osb = spool.tile([128, D], FP, tag="osb", name="osb")
        nc.vector.tensor_copy(osb[:], ps[:])
        dma(out[j * 128:(j + 1) * 128, :], osb[:])

    ctx.close()


def _build():
    if "nc" in _CACHE:
        return _CACHE["nc"]
    nc = bacc.Bacc("TRN2", target_bir_lowering=False, debug=False,
                   num_devices=N_CORES)
    names = {
        "hT": (D, TPC), "w1t": (D, D), "w2t": (D, D), "wqt": (D, D),
        "wkt": (D, D), "wpvt": (D, D), "wpot": (D, D),
        "ones128": (128, 1), "ones_r": (1, 128), "ident": (128, 128),
        "iotaL": (128, L), "simb": (1, 1), "hbmask": (1, L),
        "setm": (1, L), "lenmask": (128, 4),
        "wkeff": (D, H), "bias1": (128, NCH), "bias2": (128, NCH),
        "lng": (128, NCH), "lnb": (128, NCH), "hshift": (128, 1),
    }
    _fr = {"w1t", "w2t", "wqt", "wkt", "wpvt", "wkeff"}
    prm = {}
    for k, sh in names.items():
        dt = FR if k in _fr else (F16 if k == "wpot" else FP)
        prm[k] = nc.dram_tensor(k, list(sh), dt, kind="ExternalInput").ap()
    out = nc.dram_tensor("out", [TPC, D], FP, kind="ExternalOutput").ap()
    with tile.TileContext(nc) as tc:
        _emit(nc, tc, prm, out)
    nc.compile()
    _CACHE["nc"] = nc
    return nc


def _host_prep(inputs):
    f32 = np.float32
    hidden = np.asarray(inputs["hidden"], f32)
    lengths = np.asarray(inputs["lengths"], f32)
    consts = {
        "ones128": np.ones((128, 1), f32),
        "ones_r": np.ones((1, 128), f32),
        "ident": np.eye(128, dtype=f32),
        "iotaL": np.tile(np.arange(L, dtype=f32), (128, 1)),
        "simb": np.asarray(inputs["sim_bias"], f32).reshape(1, 1),
        "wkeff": np.ascontiguousarray(
            (np.asarray(inputs["Wpk"], np.float64).T.reshape(D, H, HD)
             * np.asarray(inputs["learned_query"],
                          np.float64).reshape(H, HD)[None]
             ).sum(-1).astype(f32)),
        "bias1": np.ascontiguousarray(
            np.asarray(inputs["b1"], f32).reshape(NCH, 128).T),
        "bias2": np.ascontiguousarray(
            np.asarray(inputs["b2"], f32).reshape(NCH, 128).T),
        "lng": np.ascontiguousarray(
            np.asarray(inputs["ln_g"], f32).reshape(NCH, 128).T),
        "lnb": np.ascontiguousarray(
            np.asarray(inputs["ln_b"], f32).reshape(NCH, 128).T),
    }
    for k, w in (("w1t", "W1"), ("w2t", "W2"), ("wqt", "Wq"), ("wkt", "Wk"),
                 ("wpvt", "Wpv")):
        consts[k] = np.ascontiguousarray(np.asarray(inputs[w], f32).T)
    consts["wpot"] = np.ascontiguousarray(
        np.asarray(inputs["Wpo"], np.float16).T)

    actual = (lengths * f32(L + 1)).astype(np.int32)
    valid = np.clip(actual - 1, 0, L)
    cut = (lengths * f32(L)).astype(np.int32)
    pos = np.arange(L)

    in_maps = []
    for c in range(N_CORES):
        b, h = c // 2, c % 2
        tok0 = h * TPC
        hT = np.ascontiguousarray(hidden[b, tok0:tok0 + TPC, :].T)
        vm = ((pos < valid[b]) & (pos < L - 1)).astype(f32)[None, :]
        st = np.zeros((1, L), f32)
        if valid[b] < L:
            st[0, valid[b]] = 1.0
        lm = np.zeros((128, 4), f32)
        for tch in range(4):
            g = tok0 + tch * 128 + np.arange(128)
            lm[:, tch] = (g < cut[b]).astype(f32)
        m = dict(consts)
        m.update({
            "hT": hT, "lenmask": lm, "hbmask": vm, "setm": st,
            "hshift": np.full((128, 1), 512.0 * h, f32),
        })
        in_maps.append(m)
    return in_maps


def kernel(**inputs):
    nc = _build()
    in_maps = _host_prep(inputs)
    res = run_bass_kernel_spmd(nc, in_maps, list(range(N_CORES)))
    out = np.empty((B, L, D), np.float32)
    for c in range(N_CORES):
        b, h = c // 2, c % 2
        out[b, h * TPC:(h + 1) * TPC, :] = res.results[c]["out"]
    return out
